# revision 16
# baseline (speedup 1.0000x reference)
"""LocalAttentionLinear Trainium2 kernel.

Math (per (b,h), N=512, D=E=64, W=256):
  S1 = q @ k^T, S2 = q_rot @ k_rot^T           [N, N]
  M1[n,m] = (m<=n) & (m>=n-255)                (causal sliding window)
  M2[n,m] = n<256 ? (n+1<=m<=n+256) : (m<=n)   (numerator-only rot window)
  D   = rowsum((S1+S2) * M1)
  out = (S1*M1 + S2*M2) @ v / D[:, None]

Device decomposition (128-chunks, i=n-chunk, j=m-chunk; scores kept
transposed S^T[m,n] so no on-chip transposes are needed anywhere):
  System A: (S1+S2)*M1 with V_aug=[v|1] -> numerator part + exact denominator.
    Heads stacked on the contraction axis (QT=[q^T;qr^T], KT=[k^T;kr^T]) so a
    single K=128 matmul produces S1+S2. Diag blocks: lower-tri mask (DVE);
    (i,i-1) full blocks: state route KVs_j = [k_j|kr_j]^T @ V_aug_j;
    (i,i-2): strict-upper mask.
  System B: S2*(M2-M1) with plain v columns only -> numerator correction with
    zero denominator impact. 7 triangular blocks via DVE masks; 3 constant
    (+/-1) blocks via KVr states on the PE.
Sharding: 64 (b,h) pairs -> 8 per core, pure data parallelism.
"""

import numpy as np
import ml_dtypes

B, H, N, D = 4, 16, 512, 64
W = 256
PER = 8  # (b,h) pairs per core
NCORES = 8
C = 128  # chunk
NCH = N // C  # 4

BF16 = ml_dtypes.bfloat16

# B-route tri blocks (i, j): elementwise (M2 - M1) masks applied on DVE.
BT = [(0, 0), (1, 1), (2, 0), (3, 1), (0, 2), (1, 2), (1, 3)]
BTGW = [4, 3]  # psum grouping: [128, 512] + [128, 384]
# B-route constant blocks routed through KVr states (no elementwise work):
#   (3,0): +Qr_3 @ KVr_0   (0,1): +Qr_0 @ KVr_1   (1,0): -Qr_1 @ KVr_0

_prog_cache = {}


def _host_masks():
    n = np.arange(N)[:, None]
    m = np.arange(N)[None, :]
    M1 = ((m <= n) & (m >= n - (W - 1))).astype(np.float32)
    M2 = np.where(n < W, (m >= n + 1) & (m <= n + W), m <= n).astype(np.float32)

    def blockT(M, i, j):
        # score blocks are transposed: psum[s, r] = S^T = S[n=Ci+r, m=Cj+s]
        return np.ascontiguousarray(M[C * i:C * i + C, C * j:C * j + C].T)

    mdiag = np.concatenate([blockT(M1, i, i) for i in range(NCH)], axis=1)
    mu = np.concatenate([blockT(M1, t + 2, t) for t in range(2)], axis=1)
    MB = M2 - M1
    mb = np.concatenate([blockT(MB, i, j) for (i, j) in BT], axis=1)
    # every MB block not in BT must be 0 or constant +-1 (state-routed)
    for i in range(NCH):
        for j in range(NCH):
            if (i, j) not in BT:
                blk = blockT(MB, i, j)
                assert blk.min() == blk.max(), (i, j)
                const = blk[0, 0]
                expect = {(3, 0): 1.0, (0, 1): 1.0, (1, 0): -1.0}.get((i, j), 0.0)
                assert const == expect, (i, j, const)
    return mdiag, mu, mb


def _build_program():
    import concourse.mybir as mybir
    import concourse.tile as tile
    import concourse.bass as bass

    dt = mybir.dt
    nc = bass.Bass()

    qt_d = nc.dram_tensor("qt", [PER, 128, N], dt.bfloat16, kind="ExternalInput")
    kt_d = nc.dram_tensor("kt", [PER, 128, N], dt.bfloat16, kind="ExternalInput")
    qrt_d = nc.dram_tensor("qrt", [PER, 64, N], dt.bfloat16, kind="ExternalInput")
    krt_d = nc.dram_tensor("krt", [PER, 64, N], dt.bfloat16, kind="ExternalInput")
    kn_d = nc.dram_tensor("kn", [PER, N, 128], dt.bfloat16, kind="ExternalInput")
    va_d = nc.dram_tensor("va", [PER, N, 65], dt.bfloat16, kind="ExternalInput")
    mdiag_d = nc.dram_tensor("mdiag", [128, NCH * C], dt.float32, kind="ExternalInput")
    mu_d = nc.dram_tensor("mu", [128, 2 * C], dt.float32, kind="ExternalInput")
    mb_d = nc.dram_tensor("mb", [128, len(BT) * C], dt.float32, kind="ExternalInput")
    out_d = nc.dram_tensor("out", [PER, N, D], dt.float32, kind="ExternalOutput")

    mult = mybir.AluOpType.mult
    Recip = mybir.ActivationFunctionType.Reciprocal
    Copy = mybir.ActivationFunctionType.Copy

    with tile.TileContext(nc) as tc:
        with (
            tc.tile_pool(name="const", bufs=1) as constp,
            tc.tile_pool(name="work", bufs=2) as workp,
            tc.tile_pool(name="psA", bufs=1, space="PSUM") as psA,
            tc.tile_pool(name="psD", bufs=2, space="PSUM") as psD,
            tc.tile_pool(name="psB", bufs=1, space="PSUM") as psB,
            tc.tile_pool(name="psO", bufs=2, space="PSUM") as psO,
        ):
            # ---- all inputs land in SBUF via one big DMA per tensor ----
            HLF = PER // 2
            qt_a = constp.tile([128, PER, N], dt.bfloat16, tag="qt_a")
            kt_a = constp.tile([128, PER, N], dt.bfloat16, tag="kt_a")
            qrt_a = constp.tile([64, PER, N], dt.bfloat16, tag="qrt_a")
            krt_a = constp.tile([64, PER, N], dt.bfloat16, tag="krt_a")
            kn_a = constp.tile([128, PER, NCH, 128], dt.bfloat16, tag="kn_a")
            va_a = constp.tile([128, PER, NCH, 65], dt.bfloat16, tag="va_a")
            for hf in range(2):
                hs = slice(HLF * hf, HLF * (hf + 1))
                nc.sync.dma_start(qt_a[:, hs], qt_d[hs].rearrange("s p n -> p s n"))
                nc.sync.dma_start(kt_a[:, hs], kt_d[hs].rearrange("s p n -> p s n"))
                nc.sync.dma_start(qrt_a[:, hs], qrt_d[hs].rearrange("s p n -> p s n"))
                nc.sync.dma_start(krt_a[:, hs], krt_d[hs].rearrange("s p n -> p s n"))
                nc.sync.dma_start(kn_a[:, hs], kn_d[hs].rearrange("s (j p) d -> p s j d", p=128))
                nc.sync.dma_start(va_a[:, hs], va_d[hs].rearrange("s (j p) e -> p s j e", p=128))
            out_a = constp.tile([128, PER, NCH, 64], dt.float32, tag="out_a")

            # Masks staged through a DVE copy so later DVE mask multiplies
            # depend on them same-engine (HW allows one sem wait per inst;
            # the PSUM-producer PE gets it).
            mdiag_g = constp.tile([128, NCH * C], dt.float32, tag="mdiag_g")
            nc.sync.dma_start(mdiag_g[:], mdiag_d[:])
            mu_g = constp.tile([128, 2 * C], dt.float32, tag="mu_g")
            nc.sync.dma_start(mu_g[:], mu_d[:])
            mb_g = constp.tile([128, len(BT) * C], dt.float32, tag="mb_g")
            nc.sync.dma_start(mb_g[:], mb_d[:])
            mdiag_s = constp.tile([128, NCH * C], dt.float32, tag="mdiag")
            nc.vector.tensor_copy(mdiag_s[:], mdiag_g[:])
            mu_s = constp.tile([128, 2 * C], dt.float32, tag="mu")
            nc.vector.tensor_copy(mu_s[:], mu_g[:])
            mb_s = constp.tile([128, len(BT) * C], dt.float32, tag="mb")
            nc.vector.tensor_copy(mb_s[:], mb_g[:])

            def w(i):
                return slice(C * i, C * i + C)

            for bh in range(PER):
                qt = qt_a[:, bh]
                kt = kt_a[:, bh]
                qrt = qrt_a[:, bh]
                krt = krt_a[:, bh]
                kn = kn_a[:, bh]
                va = va_a[:, bh]

                # ---- A diag scores: S1+S2 in one K=128 matmul per block ----
                ps_diag = psD.tile([128, NCH * C], dt.float32, tag="ps_diag")
                for i in range(NCH):
                    nc.tensor.matmul(ps_diag[:, w(i)], lhsT=kt[:, w(i)], rhs=qt[:, w(i)],
                                     start=True, stop=(i == NCH - 1),
                                     skip_group_check=True)
                pt_diag = workp.tile([128, NCH * C], dt.bfloat16, tag="pt_diag")
                nc.vector.tensor_tensor(pt_diag[:], ps_diag[:], mdiag_s[:], mult)

                # ---- A strict-upper blocks (i=t+2, j=t) ----
                ps_u = psA.tile([128, 2 * C], dt.float32, tag="ps_u")
                for t in range(2):
                    nc.tensor.matmul(ps_u[:, w(t)], lhsT=kt[:, w(t)], rhs=qt[:, w(t + 2)],
                                     start=True, stop=(t == 1), skip_group_check=True)
                pt_u = workp.tile([128, 2 * C], dt.bfloat16, tag="pt_u")
                nc.vector.tensor_tensor(pt_u[:], ps_u[:], mu_s[:], mult)

                # ---- states: A-route KVs_j (stacked, j=0..2) and B-route
                #      KVr_0, KVr_1, all in one PSUM bank [*, 5*65=325] ----
                ps_st = psA.tile([128, 5 * 65], dt.float32, tag="ps_st")
                for j in range(3):
                    nc.tensor.matmul(ps_st[:, 65 * j:65 * (j + 1)], lhsT=kn[:, j],
                                     rhs=va[:, j], start=True, stop=False,
                                     skip_group_check=True)
                for j in range(2):  # KVr_j from the kr half of kn
                    nc.tensor.matmul(ps_st[0:64, 65 * (3 + j):65 * (4 + j)],
                                     lhsT=kn[:, j, 64:128], rhs=va[:, j],
                                     start=True, stop=(j == 1),
                                     skip_group_check=True)
                st = workp.tile([128, 5 * 65], dt.bfloat16, tag="st")
                nc.scalar.activation(st[:], ps_st[:], Copy)
                stn = workp.tile([64, 65], dt.bfloat16, tag="stn")  # -KVr_0
                nc.scalar.activation(stn[:], ps_st[0:64, 65 * 3:65 * 4], Copy, scale=-1.0)

                # ---- B tri-block scores ----
                ps_b = [psB.tile([128, C * gw], dt.float32, tag=f"ps_b{g}", name=f"ps_b{g}")
                        for g, gw in enumerate(BTGW)]
                for b, (i, j) in enumerate(BT):
                    g, slot = (0, b) if b < 4 else (1, b - 4)
                    nc.tensor.matmul(ps_b[g][:, w(slot)], lhsT=krt[:, w(j)], rhs=qrt[:, w(i)],
                                     start=True, stop=(slot == BTGW[g] - 1),
                                     skip_group_check=True)
                pt_b = [workp.tile([128, C * gw], dt.bfloat16, tag=f"pt_b{g}", name=f"pt_b{g}")
                        for g, gw in enumerate(BTGW)]
                off = 0
                for g, gw in enumerate(BTGW):
                    nc.vector.tensor_tensor(pt_b[g][:], ps_b[g][:],
                                            mb_s[:, off:off + C * gw], mult)
                    off += C * gw

                # ---- accumulate out_aug[i] = [num | den], one PSUM bank ----
                ps_o = psO.tile([128, NCH * 65], dt.float32, tag="ps_o")
                last = (NCH - 1, len(BT) - 1)  # tag of final matmul
                for i in range(NCH):
                    ow = slice(65 * i, 65 * i + 65)
                    own = slice(65 * i, 65 * i + 64)
                    nc.tensor.matmul(ps_o[:, ow], lhsT=pt_diag[:, w(i)], rhs=va[:, i],
                                     start=True, stop=False, skip_group_check=True)
                    if i >= 2:
                        nc.tensor.matmul(ps_o[:, ow], lhsT=pt_u[:, w(i - 2)], rhs=va[:, i - 2],
                                         start=False, stop=False, skip_group_check=True)
                    if i >= 1:  # branch-1+2 full (i,i-1) via stacked state
                        nc.tensor.matmul(ps_o[:, ow], lhsT=qt[:, w(i)],
                                         rhs=st[:, 65 * (i - 1):65 * i],
                                         start=False, stop=False, skip_group_check=True)
                    # B const blocks via KVr states
                    if i == 3:
                        nc.tensor.matmul(ps_o[:, own], lhsT=qrt[:, w(3)],
                                         rhs=st[0:64, 65 * 3:65 * 3 + 64],
                                         start=False, stop=False, skip_group_check=True)
                    if i == 0:
                        nc.tensor.matmul(ps_o[:, own], lhsT=qrt[:, w(0)],
                                         rhs=st[0:64, 65 * 4:65 * 4 + 64],
                                         start=False, stop=False, skip_group_check=True)
                    if i == 1:
                        nc.tensor.matmul(ps_o[:, own], lhsT=qrt[:, w(1)],
                                         rhs=stn[:, 0:64],
                                         start=False, stop=False, skip_group_check=True)
                    # B tri corrections
                    bs = [b for b, (bi, _) in enumerate(BT) if bi == i]
                    for b in bs:
                        g, slot = (0, b) if b < 4 else (1, b - 4)
                        j = BT[b][1]
                        nc.tensor.matmul(ps_o[:, own], lhsT=pt_b[g][:, w(slot)],
                                         rhs=va[:, j, 0:64],
                                         start=False, stop=((i, b) == last),
                                         skip_group_check=True)

                # ---- normalize into the staging output tile ----
                ps_o3 = ps_o.rearrange("p (i e) -> p i e", e=65)
                dinv = workp.tile([128, NCH], dt.float32, tag="dinv")
                nc.vector.reciprocal(dinv[:], ps_o3[:, :, 64])
                nc.vector.tensor_tensor(out_a[:, bh], ps_o3[:, :, 0:64],
                                        dinv[:, :, None].to_broadcast((128, NCH, 64)),
                                        mult)

                if bh == HLF - 1 or bh == PER - 1:
                    hs = slice(0, HLF) if bh == HLF - 1 else slice(HLF, PER)
                    nc.sync.dma_start(
                        out_d[hs].rearrange("s (i r) e -> r s i e", r=128),
                        out_a[:, hs])

    # Legalize multi-wait instructions (TRN2 allows one sem wait per inst);
    # plain Bass + TileContext skips the Bacc compile pass that does this.
    import bass_rust
    bass_rust.move_matmul_waits_to_ldweights(nc.m)
    bass_rust.generate_event_semaphores(nc)
    return nc


def _get_prog():
    if "nc" not in _prog_cache:
        _prog_cache["nc"] = _build_program()
        _prog_cache["masks"] = _host_masks()
    return _prog_cache["nc"], _prog_cache["masks"]


def _prep_core(q, k, qr, kr, v, bhs):
    """Build the per-core input map for the 8 (b,h) pairs in bhs."""
    qt = np.empty((PER, 128, N), dtype=BF16)
    kt = np.empty((PER, 128, N), dtype=BF16)
    qrt = np.empty((PER, 64, N), dtype=BF16)
    krt = np.empty((PER, 64, N), dtype=BF16)
    kn = np.empty((PER, N, 128), dtype=BF16)
    va = np.empty((PER, N, 65), dtype=BF16)
    for s, (b, h) in enumerate(bhs):
        qt[s, :64] = q[b, h].T
        qt[s, 64:] = qr[b, h].T
        kt[s, :64] = k[b, h].T
        kt[s, 64:] = kr[b, h].T
        qrt[s] = qr[b, h].T
        krt[s] = kr[b, h].T
        kn[s, :, :64] = k[b, h]
        kn[s, :, 64:] = kr[b, h]
        va[s, :, :64] = v[b, h]
        va[s, :, 64] = 1.0
    return dict(qt=qt, kt=kt, qrt=qrt, krt=krt, kn=kn, va=va)


def kernel(q, k, q_rot, k_rot, v, _trace=False, _trace_kwargs=None):
    from concourse.bass_utils import run_bass_kernel_spmd

    q, k, q_rot, k_rot, v = (np.asarray(x, dtype=np.float32)
                             for x in (q, k, q_rot, k_rot, v))
    nc, (mdiag, mu, mb) = _get_prog()
    pairs = [(b, h) for b in range(B) for h in range(H)]
    in_maps = []
    for c in range(NCORES):
        m = _prep_core(q, k, q_rot, k_rot, v, pairs[PER * c:PER * (c + 1)])
        m["mdiag"] = mdiag
        m["mu"] = mu
        m["mb"] = mb
        in_maps.append(m)

    kw = {}
    if _trace:
        kw = dict(trace=True, trace_cores=[0], **(_trace_kwargs or {}))
    res = run_bass_kernel_spmd(nc, in_maps, core_ids=list(range(NCORES)), **kw)
    out = np.empty((B, H, N, D), dtype=np.float32)
    for c in range(NCORES):
        oc = np.asarray(res.results[c]["out"])
        for s, (b, h) in enumerate(pairs[PER * c:PER * (c + 1)]):
            out[b, h] = oc[s]
    if _trace:
        return out, res
    return out


# revision 17
# speedup vs baseline: 1.4553x; 1.4553x over previous
"""LocalAttentionLinear Trainium2 kernel.

Math (per (b,h), N=512, D=E=64, W=256):
  S1 = q @ k^T, S2 = q_rot @ k_rot^T           [N, N]
  M1[n,m] = (m<=n) & (m>=n-255)                (causal sliding window)
  M2[n,m] = n<256 ? (n+1<=m<=n+256) : (m<=n)   (numerator-only rot window)
  D   = rowsum((S1+S2) * M1)
  out = (S1*M1 + S2*M2) @ v / D[:, None]

Device decomposition (128-chunks, i=n-chunk, j=m-chunk; scores kept
transposed S^T[m,n] so no on-chip transposes are needed anywhere):
  System A: (S1+S2)*M1 with V_aug=[v|1] -> numerator part + exact denominator.
    Heads stacked on the contraction axis (QT=[q^T;qr^T], KT=[k^T;kr^T]) so a
    single K=128 matmul produces S1+S2. Diag blocks: lower-tri mask (DVE);
    (i,i-1) full blocks: state route KVs_j = [k_j|kr_j]^T @ V_aug_j;
    (i,i-2): strict-upper mask.
  System B: S2*(M2-M1) with plain v columns only -> numerator correction with
    zero denominator impact. 7 triangular blocks via DVE masks; 3 constant
    (+/-1) blocks via KVr states on the PE.
Sharding: 64 (b,h) pairs -> 8 per core, pure data parallelism.
"""

import numpy as np
import ml_dtypes

B, H, N, D = 4, 16, 512, 64
W = 256
PER = 8  # (b,h) pairs per core
NCORES = 8
C = 128  # chunk
NCH = N // C  # 4

BF16 = ml_dtypes.bfloat16

# B-route tri blocks (i, j): elementwise (M2 - M1) masks applied on DVE.
BT = [(0, 0), (1, 1), (2, 0), (3, 1), (0, 2), (1, 2), (1, 3)]
BTGW = [4, 3]  # psum grouping: [128, 512] + [128, 384]
# B-route constant blocks routed through KVr states (no elementwise work):
#   (3,0): +Qr_3 @ KVr_0   (0,1): +Qr_0 @ KVr_1   (1,0): -Qr_1 @ KVr_0

_prog_cache = {}


def _host_masks():
    n = np.arange(N)[:, None]
    m = np.arange(N)[None, :]
    M1 = ((m <= n) & (m >= n - (W - 1))).astype(np.float32)
    M2 = np.where(n < W, (m >= n + 1) & (m <= n + W), m <= n).astype(np.float32)

    def blockT(M, i, j):
        # score blocks are transposed: psum[s, r] = S^T = S[n=Ci+r, m=Cj+s]
        return np.ascontiguousarray(M[C * i:C * i + C, C * j:C * j + C].T)

    mdiag = np.concatenate([blockT(M1, i, i) for i in range(NCH)], axis=1)
    mu = np.concatenate([blockT(M1, t + 2, t) for t in range(2)], axis=1)
    MB = M2 - M1
    mb = np.concatenate([blockT(MB, i, j) for (i, j) in BT], axis=1)
    # every MB block not in BT must be 0 or constant +-1 (state-routed)
    for i in range(NCH):
        for j in range(NCH):
            if (i, j) not in BT:
                blk = blockT(MB, i, j)
                assert blk.min() == blk.max(), (i, j)
                const = blk[0, 0]
                expect = {(3, 0): 1.0, (0, 1): 1.0, (1, 0): -1.0}.get((i, j), 0.0)
                assert const == expect, (i, j, const)
    return mdiag, mu, mb


def _build_program():
    import concourse.mybir as mybir
    import concourse.tile as tile
    import concourse.bass as bass

    dt = mybir.dt
    nc = bass.Bass()

    qt_d = nc.dram_tensor("qt", [PER, 128, N], dt.bfloat16, kind="ExternalInput")
    kt_d = nc.dram_tensor("kt", [PER, 128, N], dt.bfloat16, kind="ExternalInput")
    qrt_d = nc.dram_tensor("qrt", [PER, 64, N], dt.bfloat16, kind="ExternalInput")
    krt_d = nc.dram_tensor("krt", [PER, 64, N], dt.bfloat16, kind="ExternalInput")
    kn_d = nc.dram_tensor("kn", [PER, N, 128], dt.bfloat16, kind="ExternalInput")
    va_d = nc.dram_tensor("va", [PER, N, 65], dt.bfloat16, kind="ExternalInput")
    mdiag_d = nc.dram_tensor("mdiag", [128, NCH * C], dt.float32, kind="ExternalInput")
    mu_d = nc.dram_tensor("mu", [128, 2 * C], dt.float32, kind="ExternalInput")
    mb_d = nc.dram_tensor("mb", [128, len(BT) * C], dt.float32, kind="ExternalInput")
    out_d = nc.dram_tensor("out", [PER, N, D], dt.float32, kind="ExternalOutput")

    mult = mybir.AluOpType.mult
    Recip = mybir.ActivationFunctionType.Reciprocal
    Copy = mybir.ActivationFunctionType.Copy

    with tile.TileContext(nc) as tc:
        with (
            tc.tile_pool(name="const", bufs=1) as constp,
            tc.tile_pool(name="work", bufs=2) as workp,
            tc.tile_pool(name="psA", bufs=1, space="PSUM") as psA,
            tc.tile_pool(name="psD", bufs=2, space="PSUM") as psD,
            tc.tile_pool(name="psB", bufs=1, space="PSUM") as psB,
            tc.tile_pool(name="psO", bufs=2, space="PSUM") as psO,
        ):
            # Masks staged through a DVE copy so later DVE mask multiplies
            # depend on them same-engine (HW allows one sem wait per inst;
            # the PSUM-producer PE gets it).
            mdiag_g = constp.tile([128, NCH * C], dt.float32, tag="mdiag_g")
            nc.sync.dma_start(mdiag_g[:], mdiag_d[:])
            mu_g = constp.tile([128, 2 * C], dt.float32, tag="mu_g")
            nc.sync.dma_start(mu_g[:], mu_d[:])
            mb_g = constp.tile([128, len(BT) * C], dt.float32, tag="mb_g")
            nc.sync.dma_start(mb_g[:], mb_d[:])
            mdiag_s = constp.tile([128, NCH * C], dt.float32, tag="mdiag")
            nc.vector.tensor_copy(mdiag_s[:], mdiag_g[:])
            mu_s = constp.tile([128, 2 * C], dt.float32, tag="mu")
            nc.vector.tensor_copy(mu_s[:], mu_g[:])
            mb_s = constp.tile([128, len(BT) * C], dt.float32, tag="mb")
            nc.vector.tensor_copy(mb_s[:], mb_g[:])

            # ---- all inputs land in SBUF via one big DMA per tensor ----
            HLF = PER // 4
            qt_a = constp.tile([128, PER, N], dt.bfloat16, tag="qt_a")
            kt_a = constp.tile([128, PER, N], dt.bfloat16, tag="kt_a")
            qrt_a = constp.tile([64, PER, N], dt.bfloat16, tag="qrt_a")
            krt_a = constp.tile([64, PER, N], dt.bfloat16, tag="krt_a")
            kn_a = constp.tile([128, PER, NCH, 128], dt.bfloat16, tag="kn_a")
            va_a = constp.tile([128, PER, NCH, 65], dt.bfloat16, tag="va_a")
            for hf in range(4):
                hs = slice(HLF * hf, HLF * (hf + 1))
                nc.sync.dma_start(qt_a[:, hs], qt_d[hs].rearrange("s p n -> p s n"))
                nc.sync.dma_start(kt_a[:, hs], kt_d[hs].rearrange("s p n -> p s n"))
                nc.sync.dma_start(qrt_a[:, hs], qrt_d[hs].rearrange("s p n -> p s n"))
                nc.sync.dma_start(krt_a[:, hs], krt_d[hs].rearrange("s p n -> p s n"))
                nc.sync.dma_start(kn_a[:, hs], kn_d[hs].rearrange("s (j p) d -> p s j d", p=128))
                nc.sync.dma_start(va_a[:, hs], va_d[hs].rearrange("s (j p) e -> p s j e", p=128))
            out_a = constp.tile([128, PER, NCH, 64], dt.float32, tag="out_a")

            def w(i):
                return slice(C * i, C * i + C)

            for bh in range(PER):
                qt = qt_a[:, bh]
                kt = kt_a[:, bh]
                qrt = qrt_a[:, bh]
                krt = krt_a[:, bh]
                kn = kn_a[:, bh]
                va = va_a[:, bh]

                # ---- A diag scores: S1+S2 in one K=128 matmul per block ----
                ps_diag = psD.tile([128, NCH * C], dt.float32, tag="ps_diag")
                for i in range(NCH):
                    nc.tensor.matmul(ps_diag[:, w(i)], lhsT=kt[:, w(i)], rhs=qt[:, w(i)],
                                     start=True, stop=(i == NCH - 1),
                                     skip_group_check=True)
                pt_diag = workp.tile([128, NCH * C], dt.bfloat16, tag="pt_diag")
                nc.vector.tensor_tensor(pt_diag[:], ps_diag[:], mdiag_s[:], mult)

                # ---- A strict-upper blocks (i=t+2, j=t) ----
                ps_u = psA.tile([128, 2 * C], dt.float32, tag="ps_u")
                for t in range(2):
                    nc.tensor.matmul(ps_u[:, w(t)], lhsT=kt[:, w(t)], rhs=qt[:, w(t + 2)],
                                     start=True, stop=(t == 1), skip_group_check=True)
                pt_u = workp.tile([128, 2 * C], dt.bfloat16, tag="pt_u")
                nc.vector.tensor_tensor(pt_u[:], ps_u[:], mu_s[:], mult)

                # ---- states: A-route KVs_j (stacked, j=0..2) and B-route
                #      KVr_0, KVr_1, all in one PSUM bank [*, 5*65=325] ----
                ps_st = psA.tile([128, 5 * 65], dt.float32, tag="ps_st")
                for j in range(3):
                    nc.tensor.matmul(ps_st[:, 65 * j:65 * (j + 1)], lhsT=kn[:, j],
                                     rhs=va[:, j], start=True, stop=False,
                                     skip_group_check=True)
                for j in range(2):  # KVr_j from the kr half of kn
                    nc.tensor.matmul(ps_st[0:64, 65 * (3 + j):65 * (4 + j)],
                                     lhsT=kn[:, j, 64:128], rhs=va[:, j],
                                     start=True, stop=(j == 1),
                                     skip_group_check=True)
                st = workp.tile([128, 5 * 65], dt.bfloat16, tag="st")
                nc.scalar.activation(st[:], ps_st[:], Copy)
                stn = workp.tile([64, 65], dt.bfloat16, tag="stn")  # -KVr_0
                nc.scalar.activation(stn[:], ps_st[0:64, 65 * 3:65 * 4], Copy, scale=-1.0)

                # ---- B tri-block scores ----
                ps_b = [psB.tile([128, C * gw], dt.float32, tag=f"ps_b{g}", name=f"ps_b{g}")
                        for g, gw in enumerate(BTGW)]
                for b, (i, j) in enumerate(BT):
                    g, slot = (0, b) if b < 4 else (1, b - 4)
                    nc.tensor.matmul(ps_b[g][:, w(slot)], lhsT=krt[:, w(j)], rhs=qrt[:, w(i)],
                                     start=True, stop=(slot == BTGW[g] - 1),
                                     skip_group_check=True)
                pt_b = [workp.tile([128, C * gw], dt.bfloat16, tag=f"pt_b{g}", name=f"pt_b{g}")
                        for g, gw in enumerate(BTGW)]
                off = 0
                for g, gw in enumerate(BTGW):
                    nc.vector.tensor_tensor(pt_b[g][:], ps_b[g][:],
                                            mb_s[:, off:off + C * gw], mult)
                    off += C * gw

                # ---- accumulate out_aug[i] = [num | den], one PSUM bank ----
                ps_o = psO.tile([128, NCH * 65], dt.float32, tag="ps_o")
                last = (NCH - 1, len(BT) - 1)  # tag of final matmul
                for i in range(NCH):
                    ow = slice(65 * i, 65 * i + 65)
                    own = slice(65 * i, 65 * i + 64)
                    nc.tensor.matmul(ps_o[:, ow], lhsT=pt_diag[:, w(i)], rhs=va[:, i],
                                     start=True, stop=False, skip_group_check=True)
                    if i >= 2:
                        nc.tensor.matmul(ps_o[:, ow], lhsT=pt_u[:, w(i - 2)], rhs=va[:, i - 2],
                                         start=False, stop=False, skip_group_check=True)
                    if i >= 1:  # branch-1+2 full (i,i-1) via stacked state
                        nc.tensor.matmul(ps_o[:, ow], lhsT=qt[:, w(i)],
                                         rhs=st[:, 65 * (i - 1):65 * i],
                                         start=False, stop=False, skip_group_check=True)
                    # B const blocks via KVr states
                    if i == 3:
                        nc.tensor.matmul(ps_o[:, own], lhsT=qrt[:, w(3)],
                                         rhs=st[0:64, 65 * 3:65 * 3 + 64],
                                         start=False, stop=False, skip_group_check=True)
                    if i == 0:
                        nc.tensor.matmul(ps_o[:, own], lhsT=qrt[:, w(0)],
                                         rhs=st[0:64, 65 * 4:65 * 4 + 64],
                                         start=False, stop=False, skip_group_check=True)
                    if i == 1:
                        nc.tensor.matmul(ps_o[:, own], lhsT=qrt[:, w(1)],
                                         rhs=stn[:, 0:64],
                                         start=False, stop=False, skip_group_check=True)
                    # B tri corrections
                    bs = [b for b, (bi, _) in enumerate(BT) if bi == i]
                    for b in bs:
                        g, slot = (0, b) if b < 4 else (1, b - 4)
                        j = BT[b][1]
                        nc.tensor.matmul(ps_o[:, own], lhsT=pt_b[g][:, w(slot)],
                                         rhs=va[:, j, 0:64],
                                         start=False, stop=((i, b) == last),
                                         skip_group_check=True)

                # ---- normalize into the staging output tile ----
                ps_o3 = ps_o.rearrange("p (i e) -> p i e", e=65)
                dinv = workp.tile([128, NCH], dt.float32, tag="dinv")
                nc.vector.reciprocal(dinv[:], ps_o3[:, :, 64])
                nc.vector.tensor_tensor(out_a[:, bh], ps_o3[:, :, 0:64],
                                        dinv[:, :, None].to_broadcast((128, NCH, 64)),
                                        mult)

                if (bh + 1) % HLF == 0:
                    hs = slice(bh + 1 - HLF, bh + 1)
                    nc.sync.dma_start(
                        out_d[hs].rearrange("s (i r) e -> r s i e", r=128),
                        out_a[:, hs])

    # Legalize multi-wait instructions (TRN2 allows one sem wait per inst);
    # plain Bass + TileContext skips the Bacc compile pass that does this.
    import bass_rust
    bass_rust.move_matmul_waits_to_ldweights(nc.m)
    bass_rust.generate_event_semaphores(nc)
    return nc


def _get_prog():
    if "nc" not in _prog_cache:
        _prog_cache["nc"] = _build_program()
        _prog_cache["masks"] = _host_masks()
    return _prog_cache["nc"], _prog_cache["masks"]


def _prep_core(q, k, qr, kr, v, bhs):
    """Build the per-core input map for the 8 (b,h) pairs in bhs."""
    qt = np.empty((PER, 128, N), dtype=BF16)
    kt = np.empty((PER, 128, N), dtype=BF16)
    qrt = np.empty((PER, 64, N), dtype=BF16)
    krt = np.empty((PER, 64, N), dtype=BF16)
    kn = np.empty((PER, N, 128), dtype=BF16)
    va = np.empty((PER, N, 65), dtype=BF16)
    for s, (b, h) in enumerate(bhs):
        qt[s, :64] = q[b, h].T
        qt[s, 64:] = qr[b, h].T
        kt[s, :64] = k[b, h].T
        kt[s, 64:] = kr[b, h].T
        qrt[s] = qr[b, h].T
        krt[s] = kr[b, h].T
        kn[s, :, :64] = k[b, h]
        kn[s, :, 64:] = kr[b, h]
        va[s, :, :64] = v[b, h]
        va[s, :, 64] = 1.0
    return dict(qt=qt, kt=kt, qrt=qrt, krt=krt, kn=kn, va=va)


def kernel(q, k, q_rot, k_rot, v, _trace=False, _trace_kwargs=None):
    from concourse.bass_utils import run_bass_kernel_spmd

    q, k, q_rot, k_rot, v = (np.asarray(x, dtype=np.float32)
                             for x in (q, k, q_rot, k_rot, v))
    nc, (mdiag, mu, mb) = _get_prog()
    pairs = [(b, h) for b in range(B) for h in range(H)]
    in_maps = []
    for c in range(NCORES):
        m = _prep_core(q, k, q_rot, k_rot, v, pairs[PER * c:PER * (c + 1)])
        m["mdiag"] = mdiag
        m["mu"] = mu
        m["mb"] = mb
        in_maps.append(m)

    kw = {}
    if _trace:
        kw = dict(trace=True, trace_cores=[0], **(_trace_kwargs or {}))
    res = run_bass_kernel_spmd(nc, in_maps, core_ids=list(range(NCORES)), **kw)
    out = np.empty((B, H, N, D), dtype=np.float32)
    for c in range(NCORES):
        oc = np.asarray(res.results[c]["out"])
        for s, (b, h) in enumerate(pairs[PER * c:PER * (c + 1)]):
            out[b, h] = oc[s]
    if _trace:
        return out, res
    return out
